# revision 1
# baseline (speedup 1.0000x reference)
"""DontCareLoss Trainium2 kernel.

loss = sum(per_elem) where per_elem[i,j] =
    (1 - x[i,j])^2            if j == target[i]
    0                         if j in dont_care[i] (and j != target[i])
    x[i,j]^2                  otherwise

Rewritten as:
    loss = sum(x^2)                                  # memory-bound main term
         + sum_i (1 - 2*x[i, t_i])                   # target correction
         - sum_i sum_{unique j in dc_i, j != t_i} x[i,j]^2   # dont-care correction

Sharding: data-parallel over rows, 512 rows per core on 8 cores, host adds
the 8 scalar partials.

Per core the kernel streams its [512, 10000] f32 shard through SBUF in four
[128, 10000] tiles (ACT engine Square + row-accumulate), and in parallel
gathers the 65 needed values per row (64 dont_care + 1 target) with a single
indirect DMA using host-precomputed flat int32 offsets.  Duplicate dont_care
indices are handled on the vector engine: all-pairs is_equal within each
row's 64 indices gives per-entry multiplicity m, each entry is weighted
1/m (and 0 if it equals the target), so every unique class is subtracted
exactly once.
"""

import numpy as np

import concourse.bass as bass
import concourse.tile as tile
from concourse import bacc, mybir
from concourse.bass_utils import run_bass_kernel_spmd

N, C, K = 4096, 10000, 64
NCORES = 8
ROWS = N // NCORES          # 512 rows per core
P = 128                     # SBUF partitions
T = ROWS // P               # 4 row-tiles per core
KT = K + 1                  # 64 dont_care + 1 target gather per row

F32 = mybir.dt.float32
I32 = mybir.dt.int32
AX = mybir.AxisListType
OP = mybir.AluOpType
ACT = mybir.ActivationFunctionType


def build_nc() -> bass.Bass:
    # Bacc (not raw Bass): its finalize() runs generate_event_semaphores,
    # which splits multi-sem waits into separate event-sem instructions —
    # walrus codegen allows at most one sync wait per instruction.
    nc = bacc.Bacc("TRN2", target_bir_lowering=False, debug=False)

    x = nc.declare_dram_parameter("x", [ROWS, C], F32, isOutput=False)
    offs = nc.declare_dram_parameter("offs", [P, T * KT], I32, isOutput=False)
    out = nc.declare_dram_parameter("out", [1, 1], F32, isOutput=True)

    x_tiled = x[:].rearrange("(t p) c -> t p c", p=P)   # [T, 128, C]
    x_flat = x[:].rearrange("a b -> (a b)")[:, None]    # [ROWS*C, 1]

    with tile.TileContext(nc) as tc:
        with (
            tc.tile_pool(name="xbuf", bufs=2) as xbuf,
            tc.tile_pool(name="ebuf", bufs=2) as ebuf,
            tc.tile_pool(name="small", bufs=2) as small,
            tc.tile_pool(name="persist", bufs=1) as persist,
            tc.tile_pool(name="psum", bufs=1, space="PSUM") as psum,
        ):
            # ---- gather of dont_care + target values (overlaps the stream) ----
            offs_i = persist.tile([P, T * KT], I32)
            nc.sync.dma_start(out=offs_i[:], in_=offs[:])
            offs_f = persist.tile([P, T * KT], F32)
            nc.vector.tensor_copy(out=offs_f[:], in_=offs_i[:])

            vals = persist.tile([P, T * KT], F32)
            nc.gpsimd.indirect_dma_start(
                out=vals[:],
                out_offset=None,
                in_=x_flat,
                in_offset=bass.IndirectOffsetOnAxis(ap=offs_i[:], axis=0),
            )

            # ---- main sum(x^2): stream tiles, square+accumulate on ACT ----
            # separate accum tiles per t: a shared tile would add a WAW sem
            # between ACT ops, and the ACT-accum ISA slot allows only 1 wait
            accs = [
                persist.tile([P, 1], F32, name=f"acc{t}", tag=f"acc{t}")
                for t in range(T)
            ]
            for t in range(T):
                xt = xbuf.tile([P, C], F32)
                nc.sync.dma_start(out=xt[:], in_=x_tiled[t])
                nc.scalar.activation(
                    out=xt[:], in_=xt[:], func=ACT.Square,
                    accum_out=accs[t][:],
                )

            # ---- dont-care correction with dedup, per row-tile ----
            dcsum = persist.tile([P, T], F32)     # per-tile row sums to subtract
            for t in range(T):
                o_dc = offs_f[:, t * KT : t * KT + K]          # [P, K]
                o_tg = offs_f[:, t * KT + K : t * KT + KT]     # [P, 1]
                v_dc = vals[:, t * KT : t * KT + K]            # [P, K]

                # all-pairs equality among the row's dc offsets -> multiplicity
                eq = ebuf.tile([P, K, K], F32)
                nc.vector.tensor_tensor(
                    out=eq[:],
                    in0=o_dc[:, :, None].to_broadcast([P, K, K]),
                    in1=o_dc[:, None, :].to_broadcast([P, K, K]),
                    op=OP.is_equal,
                )
                cnt = small.tile([P, K], F32)
                nc.vector.tensor_reduce(out=cnt[:], in_=eq[:], axis=AX.X, op=OP.add)
                rec = small.tile([P, K], F32)
                nc.vector.reciprocal(out=rec[:], in_=cnt[:])

                # weight 0 for entries equal to the target, else 1
                eqt = small.tile([P, K], F32)
                nc.vector.tensor_tensor(
                    out=eqt[:],
                    in0=o_dc,
                    in1=o_tg.to_broadcast([P, K]),
                    op=OP.is_equal,
                )
                w = small.tile([P, K], F32)
                nc.vector.tensor_scalar(
                    out=w[:], in0=eqt[:], scalar1=-1.0, scalar2=1.0,
                    op0=OP.mult, op1=OP.add,
                )
                wr = small.tile([P, K], F32)
                nc.vector.tensor_tensor(out=wr[:], in0=w[:], in1=rec[:], op=OP.mult)

                v2 = small.tile([P, K], F32)
                nc.vector.tensor_tensor(out=v2[:], in0=v_dc, in1=v_dc, op=OP.mult)
                v2w = small.tile([P, K], F32)
                nc.vector.tensor_tensor(out=v2w[:], in0=v2[:], in1=wr[:], op=OP.mult)
                nc.vector.tensor_reduce(
                    out=dcsum[:, t : t + 1], in_=v2w[:], axis=AX.X, op=OP.add
                )

            # ---- target correction: sum_i (1 - 2*x_t) = ROWS - 2*sum(x_t) ----
            xt_vals = vals[:].rearrange("p (t k) -> p t k", t=T)[:, :, K]  # [P, T]
            xneg = small.tile([P, T], F32)
            xt_s = persist.tile([P, 1], F32)
            nc.vector.tensor_scalar(
                out=xneg[:], in0=xt_vals, scalar1=-2.0, scalar2=None,
                op0=OP.mult, op1=OP.add, accum_out=xt_s[:],
            )

            # ---- combine per-partition, then reduce across partitions ----
            main_s = small.tile([P, 1], F32)
            nc.vector.tensor_tensor(out=main_s[:], in0=accs[0][:], in1=accs[1][:],
                                    op=OP.add)
            nc.vector.tensor_tensor(out=main_s[:], in0=main_s[:], in1=accs[2][:],
                                    op=OP.add)
            nc.vector.tensor_tensor(out=main_s[:], in0=main_s[:], in1=accs[3][:],
                                    op=OP.add)
            dc_s = small.tile([P, 1], F32)
            nc.vector.tensor_reduce(out=dc_s[:], in_=dcsum[:], axis=AX.X, op=OP.add)

            tot = small.tile([P, 1], F32)
            nc.vector.tensor_tensor(out=tot[:], in0=main_s[:], in1=dc_s[:],
                                    op=OP.subtract)
            tot2 = small.tile([P, 1], F32)
            nc.vector.tensor_tensor(out=tot2[:], in0=tot[:], in1=xt_s[:], op=OP.add)

            ones = persist.tile([P, 1], F32)
            nc.vector.memset(ones[:], 1.0)
            ps = psum.tile([1, 1], F32)
            nc.tensor.matmul(out=ps[:], lhsT=tot2[:], rhs=ones[:],
                             start=True, stop=True)

            # + ROWS (the constant 1 per row from (1-x_t)^2 expansion)
            fin = small.tile([1, 1], F32)
            nc.vector.tensor_scalar_add(out=fin[:], in0=ps[:], scalar1=float(ROWS))
            nc.sync.dma_start(out=out[:], in_=fin[:])

    nc.finalize()
    return nc


_NC = None


def _get_nc():
    global _NC
    if _NC is None:
        _NC = build_nc()
    return _NC


def make_in_maps(input, target, dont_care):
    input = np.asarray(input, dtype=np.float32)
    target = np.asarray(target)
    dont_care = np.asarray(dont_care)
    in_maps = []
    for c in range(NCORES):
        sl = slice(c * ROWS, (c + 1) * ROWS)
        xs = np.ascontiguousarray(input[sl])                      # [ROWS, C]
        dc = dont_care[sl].astype(np.int32)                       # [ROWS, K]
        tg = target[sl].astype(np.int32)[:, None]                 # [ROWS, 1]
        idx = np.concatenate([dc, tg], axis=1)                    # [ROWS, KT]
        off = np.arange(ROWS, dtype=np.int32)[:, None] * C + idx  # flat offsets
        # device layout: [P, T*KT], col t*KT+k = row t*P+p, entry k
        off_dev = np.ascontiguousarray(
            off.reshape(T, P, KT).transpose(1, 0, 2).reshape(P, T * KT)
        )
        in_maps.append({"x": xs, "offs": off_dev})
    return in_maps


def kernel(input, target, dont_care):
    nc = _get_nc()
    in_maps = make_in_maps(input, target, dont_care)
    res = run_bass_kernel_spmd(nc, in_maps, core_ids=list(range(NCORES)))
    partials = [r["out"][0, 0] for r in res.results]
    return np.float32(np.sum(np.asarray(partials, dtype=np.float64)))



# revision 23
# speedup vs baseline: 1.1633x; 1.1633x over previous
"""DontCareLoss Trainium2 kernel.

loss = sum(per_elem) where per_elem[i,j] =
    (1 - x[i,j])^2            if j == target[i]
    0                         if j in dont_care[i] (and j != target[i])
    x[i,j]^2                  otherwise

Rewritten as:
    loss = sum(x^2)                                  # memory-bound main term
         + sum_i (1 - 2*x[i, t_i])                   # target correction
         - sum_i sum_k wq[i,k] * x[i, dc_ik]^2       # dont-care correction

where wq[i,k] = 1/multiplicity(dc_ik within row i) (0 if dc_ik == t_i) is
precomputed on the HOST from the integer index tensors, so every unique
dont-care class j != t_i is subtracted exactly once.  This removes all the
dedup (all-pairs is_equal) work from the device's vector engine.

Sharding: data-parallel over rows, 512 rows per core on 8 cores, host adds
the 8 scalar partials.

Per core the kernel streams its [512, 10000] f32 shard through SBUF as 19
column-chunks across 4 128-row tiles, each chunk a separate DMA into its own
persistent SBUF buffer (all issued up front, no write-after-read waits),
squared+row-accumulated as soon as it lands.  Full tiles run on the ACT
engine; the last tile tapers (2500,2500,1666,1666,833,522,313) and
alternates between ACT and the otherwise-idle vector engine so the compute
tail after the last DMA byte is only ~1us.  In parallel the 65 needed
values per row (64 dont_care + 1 target) are gathered with one indirect DMA
using host-precomputed flat int32 offsets, and the correction terms are
folded in with 4 small vector ops using the host dedup weights, all hidden
under the stream.  Final cross-partition reduce is a matmul with ones.
"""

import numpy as np

import concourse.bass as bass
import concourse.tile as tile
from concourse import bacc, mybir
from concourse.bass_utils import run_bass_kernel_spmd

N, C, K = 4096, 10000, 64
NCORES = 8
ROWS = N // NCORES          # 512 rows per core
P = 128                     # SBUF partitions
T = ROWS // P               # 4 row-tiles per core
KT = K + 1                  # 64 dont_care + 1 target gather per row

# column chunking per row-tile: full tiles stream in 2500-col chunks on the
# ACT engine ("S"); the last row-tile tapers and alternates between ACT and
# the (otherwise idle) vector engine ("V") so the post-stream compute tail
# is only the final 313-col chunk
FULL_CHUNKS = [(2500, "S"), (2500, "S"), (2500, "S"), (2500, "S")]
LAST_CHUNKS = [(2500, "S"), (2500, "V"), (1666, "S"), (1666, "V"),
               (833, "S"), (522, "V"), (313, "S")]
assert sum(w for w, _ in FULL_CHUNKS) == C
assert sum(w for w, _ in LAST_CHUNKS) == C

F32 = mybir.dt.float32
I32 = mybir.dt.int32
AX = mybir.AxisListType
OP = mybir.AluOpType
ACT = mybir.ActivationFunctionType


def build_nc() -> bass.Bass:
    # Bacc (not raw Bass): its finalize() runs generate_event_semaphores,
    # which splits multi-sem waits into separate event-sem instructions —
    # walrus codegen allows at most one sync wait per instruction.
    nc = bacc.Bacc("TRN2", target_bir_lowering=False, debug=False)

    x = nc.declare_dram_parameter("x", [ROWS, C], F32, isOutput=False)
    offs = nc.declare_dram_parameter("offs", [P, T * KT], I32, isOutput=False)
    wq = nc.declare_dram_parameter("wq", [P, T * KT], F32, isOutput=False)
    out = nc.declare_dram_parameter("out", [1, 1], F32, isOutput=True)

    x_tiled = x[:].rearrange("(t p) c -> t p c", p=P)   # [T, 128, C]
    x_flat = x[:].rearrange("a b -> (a b)")[:, None]    # [ROWS*C, 1]

    # (tile index, col start, width, engine) for every chunk, in stream order
    chunks = []
    for t in range(T):
        c0 = 0
        for w, eng in (FULL_CHUNKS if t < T - 1 else LAST_CHUNKS):
            chunks.append((t, c0, w, eng))
            c0 += w

    with tile.TileContext(nc) as tc:
        with (
            tc.tile_pool(name="persist", bufs=1) as persist,
            tc.tile_pool(name="psum", bufs=1, space="PSUM") as psum,
        ):
            # ---- main stream: issue ALL chunk DMAs up front on sync ----
            xb = [
                persist.tile([P, w], F32, name=f"xc{i}", tag=f"xc{i}")
                for i, (_, _, w, _) in enumerate(chunks)
            ]
            for i, (t, c0, w, _) in enumerate(chunks):
                nc.sync.dma_start(out=xb[i][:], in_=x_tiled[t][:, c0 : c0 + w])

            # gather offsets + dedup weights issued from the scalar engine
            # (idle until the first chunk lands) so they don't delay the
            # stream DMA issues on sync
            offs_i = persist.tile([P, T * KT], I32)
            nc.scalar.dma_start(out=offs_i[:], in_=offs[:])
            wq_t = persist.tile([P, T * KT], F32)
            nc.scalar.dma_start(out=wq_t[:], in_=wq[:])

            # gather of dont_care + target values (overlaps the stream)
            vals = persist.tile([P, T * KT], F32)
            nc.gpsimd.indirect_dma_start(
                out=vals[:],
                out_offset=None,
                in_=x_flat,
                in_offset=bass.IndirectOffsetOnAxis(ap=offs_i[:], axis=0),
            )

            ones = persist.tile([P, 1], F32)
            nc.vector.memset(ones[:], 1.0)

            # ---- sum(x^2): square+row-accumulate each chunk, "S" chunks
            # on the ACT engine, "V" chunks on the vector engine ----
            # separate accum tiles: a shared tile would add a WAW sem
            # between ACT ops, and the ACT-accum ISA slot allows only 1 wait
            accs = [
                persist.tile([P, 1], F32, name=f"acc{i}", tag=f"acc{i}")
                for i in range(len(chunks))
            ]
            for i, (_, _, _, eng) in enumerate(chunks):
                if eng == "S":
                    nc.scalar.activation(
                        out=xb[i][:], in_=xb[i][:], func=ACT.Square,
                        accum_out=accs[i][:],
                    )

            # ---- corrections from the gathered values (all tiny) ----
            # dc: s = -sum_k wq*v^2 ; target: tlin = sum_t (1 - 2*v_t)
            m = persist.tile([P, T * KT], F32)
            nc.vector.tensor_tensor(out=m[:], in0=vals[:], in1=wq_t[:], op=OP.mult)
            scr = persist.tile([P, T * KT], F32)
            nc.vector.tensor_tensor(out=scr[:], in0=m[:], in1=vals[:], op=OP.mult)
            dcs = persist.tile([P, 1], F32)
            nc.vector.tensor_reduce(out=dcs[:], in_=scr[:], axis=AX.X, op=OP.add)
            v_tgt = vals[:].rearrange("p (t k) -> p t k", t=T)[:, :, K]  # [P, T]
            ts4 = persist.tile([P, T], F32)
            tlin = persist.tile([P, 1], F32)
            nc.vector.tensor_scalar(
                out=ts4[:], in0=v_tgt, scalar1=-2.0, scalar2=1.0,
                op0=OP.mult, op1=OP.add, accum_out=tlin[:],
            )
            s = persist.tile([P, 1], F32)
            nc.vector.tensor_tensor(out=s[:], in0=tlin[:], in1=dcs[:],
                                    op=OP.subtract)

            # ---- running sum of the chunk accumulators (hidden under
            # the stream; only the last add is on the critical tail).
            # "V" chunks compute their square+accum here on the vector
            # engine, in stream order, in-place like the ACT chunks ----
            for i, (_, _, _, eng) in enumerate(chunks):
                if eng == "V":
                    nc.vector.scalar_tensor_tensor(
                        out=xb[i][:], in0=xb[i][:], scalar=1.0, in1=xb[i][:],
                        op0=OP.mult, op1=OP.mult, accum_out=accs[i][:],
                    )
                nc.vector.tensor_tensor(out=s[:], in0=s[:], in1=accs[i][:],
                                        op=OP.add)

            # ---- reduce across partitions via matmul with ones ----
            ps = psum.tile([1, 1], F32)
            nc.tensor.matmul(out=ps[:], lhsT=s[:], rhs=ones[:],
                             start=True, stop=True)
            fin = persist.tile([1, 1], F32)
            nc.vector.tensor_copy(out=fin[:], in_=ps[:])
            nc.sync.dma_start(out=out[:], in_=fin[:])

    nc.finalize()
    return nc


_NC = None


def _get_nc():
    global _NC
    if _NC is None:
        _NC = build_nc()
    return _NC


def make_in_maps(input, target, dont_care):
    input = np.asarray(input, dtype=np.float32)
    target = np.asarray(target)
    dont_care = np.asarray(dont_care)
    in_maps = []
    for c in range(NCORES):
        sl = slice(c * ROWS, (c + 1) * ROWS)
        xs = np.ascontiguousarray(input[sl])                      # [ROWS, C]
        dc = dont_care[sl].astype(np.int32)                       # [ROWS, K]
        tg = target[sl].astype(np.int32)[:, None]                 # [ROWS, 1]
        idx = np.concatenate([dc, tg], axis=1)                    # [ROWS, KT]
        off = np.arange(ROWS, dtype=np.int32)[:, None] * C + idx  # flat offsets
        # dedup weights: each dc entry weighted 1/multiplicity within its
        # row (so duplicate classes subtract once), 0 if it equals the
        # target; the appended target column gets weight 0
        mult = (dc[:, :, None] == dc[:, None, :]).sum(axis=2)     # [ROWS, K]
        w = (1.0 / mult) * (dc != tg)                             # [ROWS, K]
        wfull = np.concatenate(
            [w.astype(np.float32), np.zeros((ROWS, 1), np.float32)], axis=1
        )                                                         # [ROWS, KT]
        # device layout: [P, T*KT], col t*KT+k = row t*P+p, entry k
        off_dev = np.ascontiguousarray(
            off.reshape(T, P, KT).transpose(1, 0, 2).reshape(P, T * KT)
        )
        wq_dev = np.ascontiguousarray(
            wfull.reshape(T, P, KT).transpose(1, 0, 2).reshape(P, T * KT)
        )
        in_maps.append({"x": xs, "offs": off_dev, "wq": wq_dev})
    return in_maps


def kernel(input, target, dont_care):
    nc = _get_nc()
    in_maps = make_in_maps(input, target, dont_care)
    res = run_bass_kernel_spmd(nc, in_maps, core_ids=list(range(NCORES)))
    partials = [r["out"][0, 0] for r in res.results]
    return np.float32(np.sum(np.asarray(partials, dtype=np.float64)))
